# revision 1
# baseline (speedup 1.0000x reference)
"""CRF-as-RNN mean-field kernel for Trainium2 (Bass/Tile), 8-core SPMD.

Strategy:
- Shard 2 images x 4 row-strips across 8 cores. Each core gets 84 rows
  (64 owned + halo); 5 mean-field iterations shrink the valid region by
  2 rows/iter, so no inter-core communication is needed at all.
- On-chip layout: partitions = 6 row-groups x 21 channels = 126; free dim
  = 14 rows x 256 cols (+2-row/-col halos for in-tile shifted reads:
  18 row-slots x 260 col-slots). Image-boundary zero padding is realized
  by statically-zero halo slots; intra-core group halos are refreshed
  once per iteration with two SBUF->SBUF DMAs.
- The 5x5 spatial gaussian (sigma=0.1) is a numerical delta in f32, so
  sp == q; it is folded into the center-tap constant.
- Bilateral 24-tap MAC runs on DVE in fp16 (2x mode where aligned),
  using 12 unique weight maps (opposite taps share maps by symmetry).
- Softmax runs chunked through PSUM: z = logits - compat-transform via
  PE matmuls; exp/ln on ACT; normalization via the exp(z - lnD) trick
  (lnD broadcast back into PSUM by a mask matmul) - no division needed.
- Bilateral color weights are precomputed once: diff/square on DVE/ACT,
  3-channel reduction + 21-channel broadcast via PE mask matmuls,
  exp(-50*d^2 + ln(spatial)) on ACT.
"""

import math
import sys
from contextlib import ExitStack

import numpy as np

sys.path.insert(0, "/opt/trn_rl_repo")

# ---------------- problem constants (hardcoded per contract) ----------------
B, C, H, W = 2, 21, 256, 256
G, RG = 6, 14                  # row groups per strip, rows per group
P = G * C                      # 126 partitions
F = RG * W                     # 3584 free elems (real pixels per partition)
NT, NV = 18, 260               # q/w tile row slots (-2..15), col slots (-2..257)
IU, IV = 22, 264               # img tile row slots (-4..17), col slots (-4..259)
STARTS = [0, 54, 118, 172]     # strip start rows
OWN = [(0, 64), (10, 74), (10, 74), (20, 84)]  # owned local-row range per strip
NUM_ITERS = 5
NCH, CH = 7, 512               # softmax chunks (512 px = 2 rows)
NPC, CP = 10, 468              # w-precompute chunks over NT*NV=4680

# spatial gaussian (5x5, sigma=5), normalized
_ax = np.arange(5, dtype=np.float64) - 2
_xx, _yy = np.meshgrid(_ax, _ax, indexing="ij")
_g = np.exp(-(_xx**2 + _yy**2) / (2 * 5.0**2))
SW = (_g / _g.sum()).astype(np.float64)
WC = float(SW[2, 2])           # center weight (spatial only; color=1 at center)
# 12 unique taps (positive half-window); opposite taps share weight maps
TAPS = [(0, 1), (0, 2), (1, -2), (1, -1), (1, 0), (1, 1), (1, 2),
        (2, -2), (2, -1), (2, 0), (2, 1), (2, 2)]

_BASS_CACHE = {}


def _build_bass():
    import concourse.bass as bass
    import concourse.mybir as mybir
    from concourse import tile

    f32 = mybir.dt.float32
    f16 = mybir.dt.float16
    AF = mybir.ActivationFunctionType
    OP = mybir.AluOpType

    nc = bass.Bass("TRN2", target_bir_lowering=False, debug=False,
                   enable_asserts=False)

    lg_d = nc.dram_tensor("lg", [P, F], f32, kind="ExternalInput")
    img_d = nc.dram_tensor("img", [18, IU * IV], f32, kind="ExternalInput")
    mneg_d = nc.dram_tensor("mneg", [P, P], f16, kind="ExternalInput")
    iden_d = nc.dram_tensor("iden", [P, P], f32, kind="ExternalInput")
    onesd_d = nc.dram_tensor("onesd", [P, G], f16, kind="ExternalInput")
    chmask_d = nc.dram_tensor("chmask", [18, G], f16, kind="ExternalInput")
    bneg_d = nc.dram_tensor("bneg", [G, P], f32, kind="ExternalInput")
    bpos_d = nc.dram_tensor("bpos", [G, P * 12], f32, kind="ExternalInput")
    qout_d = nc.dram_tensor("qout", [P, F], f32, kind="ExternalOutput")

    with tile.TileContext(nc) as tc, ExitStack() as ctx:
        const_pool = ctx.enter_context(tc.tile_pool(name="const", bufs=1))
        main_pool = ctx.enter_context(tc.tile_pool(name="main", bufs=1))
        w_pool = ctx.enter_context(tc.tile_pool(name="wmaps", bufs=1))

        mneg_t = const_pool.tile([P, P], f16, tag="mneg")
        nc.sync.dma_start(mneg_t[:], mneg_d.ap())
        iden_t = const_pool.tile([P, P], f32, tag="iden")
        nc.sync.dma_start(iden_t[:], iden_d.ap())
        onesd_t = const_pool.tile([P, G], f16, tag="onesd")
        nc.sync.dma_start(onesd_t[:], onesd_d.ap())
        chmask_t = const_pool.tile([18, G], f16, tag="chmask")
        nc.sync.dma_start(chmask_t[:], chmask_d.ap())
        bneg_t = const_pool.tile([G, P], f32, tag="bneg")
        nc.sync.dma_start(bneg_t[:], bneg_d.ap())
        bpos_t = const_pool.tile([G, P * 12], f32, tag="bpos")
        nc.sync.dma_start(bpos_t[:], bpos_d.ap())

        # Absorber matmuls: each PE matmul can carry only ~1 sync wait
        # beyond its own-engine wait, so pre-observe every stationary's DMA
        # queue with a 2-column dummy matmul (self-referential rhs => the
        # dummy itself waits on exactly one DMA sem).
        with tc.tile_pool(name="scrp", bufs=1, space="PSUM") as scrp:
            scr = scrp.tile([G, 2], f32, tag="scr")
            nc.tensor.matmul(scr[:1, :], mneg_t[:, 0:1], mneg_t[:, 0:2],
                             start=True, stop=True)
            nc.tensor.matmul(scr[:1, :], iden_t[:, 0:1], iden_t[:, 0:2],
                             start=True, stop=True)
            nc.tensor.matmul(scr[:, :], onesd_t[:], onesd_t[:, 0:2],
                             start=True, stop=True)
            nc.tensor.matmul(scr[:, :], chmask_t[:], chmask_t[:, 0:2],
                             start=True, stop=True)
            nc.tensor.matmul(scr[:1, :], bneg_t[:, 0:1], bneg_t[:, 0:2],
                             start=True, stop=True)
            nc.tensor.matmul(scr[:1, :], bpos_t[:, 0:1], bpos_t[:, 0:2],
                             start=True, stop=True)

        q_t = main_pool.tile([P, NT * NV], f16, tag="q")
        nc.vector.memset(q_t[:], 0.0)
        q3 = q_t[:].rearrange("p (t v) -> p t v", v=NV)


        w_tiles = [w_pool.tile([P, NT * NV], f16, tag=f"w{i}", name=f"w{i}")
                   for i in range(len(TAPS))]

        zps_pool = ctx.enter_context(tc.tile_pool(name="zps", bufs=3,
                                                  space="PSUM"))
        dps_pool = ctx.enter_context(tc.tile_pool(name="dps", bufs=1,
                                                  space="PSUM"))

        # ---------------- w-map precompute ----------------
        with tc.tile_pool(name="pre", bufs=1) as prep, \
             tc.tile_pool(name="pre2", bufs=3) as prep2, \
             tc.tile_pool(name="psp", bufs=2, space="PSUM") as psp, \
             tc.tile_pool(name="psw", bufs=2, space="PSUM") as psw:
            img_t = prep.tile([18, IU * IV], f32, tag="img")
            nc.sync.dma_start(img_t[:], img_d.ap())
            img3 = img_t[:].rearrange("p (u v) -> p u v", v=IV)
            diff_t = prep.tile([18, NT * NV], f16, tag="diff")
            diff3 = diff_t[:].rearrange("p (t v) -> p t v", v=NV)
            sq_t = prep.tile([18, NT * NV], f16, tag="sq")

            for ki, (dy, dx) in enumerate(TAPS):
                nc.vector.tensor_sub(
                    diff3[:, 0:NT, 0:NV],
                    img3[:, 2 + dy:2 + dy + NT, 2 + dx:2 + dx + NV],
                    img3[:, 2:2 + NT, 2:2 + NV],
                )
                nc.scalar.square(sq_t[:], diff_t[:])
                for cc in range(NPC):
                    sl = slice(cc * CP, (cc + 1) * CP)
                    d2_ps = psp.tile([G, CP], f32, tag="d2")
                    nc.tensor.matmul(d2_ps[:], chmask_t[:],
                                     sq_t[:, sl], start=True, stop=True)
                    e6 = prep2.tile([G, CP], f32, tag="e6")
                    nc.scalar.activation(e6[:], d2_ps[:], AF.Exp,
                                         scale=-50.0)
                    w_ps = psw.tile([P, CP], f32, tag="wps")
                    nc.tensor.matmul(w_ps[:],
                                     bpos_t[:, ki * P:(ki + 1) * P], e6[:],
                                     start=True, stop=True)
                    nc.scalar.copy(w_tiles[ki][:, sl], w_ps[:])

        # ---------------- iteration tiles ----------------
        post_pool = ctx.enter_context(tc.tile_pool(name="post", bufs=1))
        lg_t = post_pool.tile([P, F], f32, tag="lg")
        nc.sync.dma_start(lg_t[:], lg_d.ap())
        lg2_t = post_pool.tile([P, F], f32, tag="lg2")
        nc.scalar.copy(lg2_t[:], lg_t[:])  # ACT-owned copy for PE reads
        acc_t = post_pool.tile([P, F], f16, tag="acc")
        acc3 = acc_t[:].rearrange("p (r x) -> p r x", x=W)
        tmp_pool = ctx.enter_context(tc.tile_pool(name="tmp", bufs=2))
        e_pool = ctx.enter_context(tc.tile_pool(name="E", bufs=2))
        ln_pool = ctx.enter_context(tc.tile_pool(name="ln", bufs=2))

        def softmax_pass(with_s: bool, last: bool):
            for c in range(NCH):
                sl = slice(c * CH, (c + 1) * CH)
                z_ps = zps_pool.tile([P, CH], f32, tag="z")
                if with_s:
                    nc.tensor.matmul(z_ps[:], mneg_t[:], acc_t[:, sl],
                                     start=True, stop=False)
                    nc.tensor.matmul(z_ps[:], iden_t[:], lg2_t[:, sl],
                                     start=False, stop=False,
                                     skip_group_check=True)
                else:
                    nc.tensor.matmul(z_ps[:], iden_t[:], lg2_t[:, sl],
                                     start=True, stop=False,
                                     skip_group_check=True)
                e_t = e_pool.tile([P, CH], f16, tag="E")
                nc.scalar.activation(e_t[:], z_ps[:], AF.Exp)
                d_ps = dps_pool.tile([G, CH], f32, tag="D")
                nc.tensor.matmul(d_ps[:], onesd_t[:], e_t[:],
                                 start=True, stop=True)
                ln_t = ln_pool.tile([G, CH], f32, tag="ln")
                nc.scalar.activation(ln_t[:], d_ps[:], AF.Ln)
                nc.tensor.matmul(z_ps[:], bneg_t[:], ln_t[:],
                                 start=False, stop=True,
                                 skip_group_check=True)
                z3 = z_ps[:].rearrange("p (r x) -> p r x", x=W)
                if last:
                    lg3 = lg_t[:].rearrange("p (r x) -> p r x", x=W)
                    nc.scalar.activation(lg3[:, 2 * c:2 * c + 2, 0:W],
                                         z3, AF.Exp)
                else:
                    nc.scalar.activation(
                        q3[:, 2 + 2 * c:4 + 2 * c, 2:2 + W], z3, AF.Exp)

        softmax_pass(with_s=False, last=False)   # q0 = softmax(logits)

        for it in range(NUM_ITERS):
            last = it == NUM_ITERS - 1
            # refresh intra-core group halos (2 SBUF->SBUF DMAs)
            nc.sync.dma_start(q3[21:126, 0:2, 0:NV], q3[0:105, 14:16, 0:NV])
            nc.sync.dma_start(q3[0:105, 16:18, 0:NV], q3[21:126, 2:4, 0:NV])

            # bilateral: 24 taps = 12 unique maps x {gather, scatter-sym}
            first = True
            for ki, (dy, dx) in enumerate(TAPS):
                w3 = w_tiles[ki][:].rearrange("p (t v) -> p t v", v=NV)
                for (qdy, qdx, wdy, wdx) in ((dy, dx, 0, 0),
                                             (-dy, -dx, -dy, -dx)):
                    q_ap = q3[:, 2 + qdy:2 + qdy + RG, 2 + qdx:2 + qdx + W]
                    w_ap = w3[:, 2 + wdy:2 + wdy + RG, 2 + wdx:2 + wdx + W]
                    if first:
                        nc.vector.tensor_mul(acc3[:, 0:RG, 0:W], q_ap, w_ap)
                        first = False
                    else:
                        t = tmp_pool.tile([P, F], f16, tag="tmp")
                        t3 = t[:].rearrange("p (r x) -> p r x", x=W)
                        nc.vector.tensor_mul(t3[:, 0:RG, 0:W], q_ap, w_ap)
                        nc.vector.tensor_add(acc_t[:], acc_t[:], t[:])

            # acc = q*(1+wc) + acc   (spatial delta-conv fold, in place)
            nc.vector.scalar_tensor_tensor(
                acc3[:, 0:RG, 0:W], q3[:, 2:2 + RG, 2:2 + W],
                float(1.0 + WC), acc3[:, 0:RG, 0:W],
                OP.mult, OP.add)

            softmax_pass(with_s=True, last=last)

        nc.sync.dma_start(qout_d.ap(), lg_t[:])

    _legalize_matmul_waits(nc, mybir)
    return nc


def _legalize_matmul_waits(nc, mybir, max_waits=2):
    """TRN2 ISA sync-wait structs hold few waits per instruction (2 for PE
    matmult/NoOp, 1 for DVE TensorTensor, ...); codegen aborts on more.
    Move excess waits onto InstNoOps (1 wait each) inserted right before
    on the same engine (adjacent => identical blocking semantics)."""
    cap = {}
    for f in nc.m.functions:
        for blk in f.blocks:
            insts = blk.instructions
            out = []
            changed = False
            for i in insts:
                si = getattr(i, "sync_info", None)
                eng = getattr(i, "engine", None)
                max_waits = cap.get(type(i).__name__, 1)
                if (si is not None and eng is not None
                        and len(si.on_wait) > max_waits):
                    waits = list(si.on_wait)
                    keep, move = [], []
                    for w in waits:
                        if "PE" in w.ant_name and len(keep) < max_waits:
                            keep.append(w)
                        else:
                            move.append(w)
                    while len(keep) < max_waits and move:
                        keep.append(move.pop())
                    nop_cap = cap.get("InstNoOp", 1)
                    while move:
                        grp, move = move[:nop_cap], move[nop_cap:]
                        nop = mybir.InstNoOp(
                            name=nc.get_next_instruction_name(),
                            engine=eng, ins=[], outs=[])
                        nop.sync_info = mybir.SyncInfo(on_wait=grp,
                                                       on_update=[])
                        out.append(nop)
                    i.sync_info = mybir.SyncInfo(
                        on_wait=keep, on_update=list(si.on_update))
                    changed = True
                out.append(i)
            if changed:
                blk.instructions = out


def _prep_shards(logits, img, compat):
    """Host-side shard prep -> list of 8 in_maps."""
    mneg = np.kron(np.eye(G), -compat.T.astype(np.float64)).astype(np.float16)
    iden = np.eye(P, dtype=np.float32)
    onesd = np.kron(np.eye(G), np.ones((C, 1))).astype(np.float16)
    chmask = np.kron(np.eye(G), np.ones((3, 1))).astype(np.float16)
    bneg = np.kron(np.eye(G), -np.ones((1, C))).astype(np.float32)
    bpos = np.concatenate(
        [np.kron(np.eye(G), float(SW[2 + dy, 2 + dx]) * np.ones((1, C)))
         for (dy, dx) in TAPS], axis=1).astype(np.float32)

    in_maps = []
    for core in range(8):
        b, j = divmod(core, 4)
        s = STARTS[j]
        lg = logits[b, :, s:s + 84, :].reshape(C, G, RG, W)
        lg = np.ascontiguousarray(
            lg.transpose(1, 0, 2, 3).reshape(P, F)).astype(np.float32)
        im = np.zeros((G, 3, IU, IV), np.float32)
        for g in range(G):
            base = s + g * RG - 4
            u0, u1 = max(0, -base), min(IU, H - base)
            im[g, :, u0:u1, 4:4 + W] = img[b, :, base + u0:base + u1, :]
        im = im.reshape(18, IU * IV)
        in_maps.append({
            "lg": lg, "img": np.ascontiguousarray(im),
            "mneg": mneg, "iden": iden, "onesd": onesd,
            "chmask": chmask, "bneg": bneg, "bpos": bpos,
        })
    return in_maps


def kernel(**inputs):
    logits = np.asarray(inputs["logits"], dtype=np.float32)
    img = np.asarray(inputs["img"], dtype=np.float32)
    compat = np.asarray(inputs["compat_mat"], dtype=np.float32)

    from concourse.bass_utils import run_bass_kernel_spmd

    if "nc" not in _BASS_CACHE:
        _BASS_CACHE["nc"] = _build_bass()
    nc = _BASS_CACHE["nc"]

    in_maps = _prep_shards(logits, img, compat)
    res = run_bass_kernel_spmd(nc, in_maps, core_ids=list(range(8)))
    _BASS_CACHE["last_result"] = res

    out = np.zeros((B, C, H, W), np.float32)
    for core in range(8):
        b, j = divmod(core, 4)
        s = STARTS[j]
        lo, hi = OWN[j]
        qc = res.results[core]["qout"].reshape(G, C, RG, W)
        qc = qc.transpose(1, 0, 2, 3).reshape(C, 84, W)
        out[b, :, s + lo:s + hi, :] = qc[:, lo:hi, :]
    return out



# revision 9
# speedup vs baseline: 1.4249x; 1.4249x over previous
"""CRF-as-RNN mean-field kernel for Trainium2 (Bass/Tile), 8-core SPMD.

Strategy (v2):
- Shard 2 images x 4 row-strips across 8 cores; 84 rows/core (64 owned +
  halo), 5 mean-field iterations shrink validity, no inter-core comms.
- Partitions = 6 row-groups x 21 channels = 126; free = 14 rows x 256 cols
  (+2 halos: 18 x 260 slots).
- The 5x5 spatial gaussian (sigma=0.1) is a delta => sp == q, folded into
  a second stationary mneg2 = (1+wc) * mneg applied to q via PE.
- Bilateral: 24 shifted products on DVE (fp16 2x), accumulated ON THE PE
  via mneg x t_k matmuls into 7 persistent PSUM z-banks (compat transform
  is linear). No DVE adds, no SBUF acc tile, f32 accumulation.
- Softmax: z-banks already hold logits + pairwise (logits fed as f32
  bitcast to f32r: full PE rate at 512 cols). exp/ln on ACT; lnD
  broadcast back via bneg (f32r) matmul; final exp writes q (fp16) or the
  f32 output tile.
- w-precompute, column-sharded: partitions (group, rgb, quarter) = 72;
  per tap: DVE diff, DVE square, PE rgb-reduce -> [24, 390] PSUM x3,
  ACT exp(-50*d2 + ln(spatial)) -> compact [24, 12*1170] fp16; then 84
  strided-partition DMAs replicate rows to the 21 channel partitions
  (w_all [126, 12*4680] fp16).
"""

import math
import sys
from contextlib import ExitStack

import numpy as np

sys.path.insert(0, "/opt/trn_rl_repo")

# ---------------- problem constants (hardcoded per contract) ----------------
B, C, H, W = 2, 21, 256, 256
G, RG = 6, 14                  # row groups per strip, rows per group
P = G * C                      # 126 partitions
F = RG * W                     # 3584 free elems per partition
NT, NV = 18, 260               # q/w map slots: rows -2..15, cols -2..257
STARTS = [0, 54, 118, 172]     # strip start rows
OWN = [(0, 64), (10, 74), (10, 74), (20, 84)]  # owned local-row range
NUM_ITERS = 5
NCH, CH = 7, 512               # softmax chunks (512 px = 2 rows)
NQ, QW = 4, 65                 # w-precompute col quarters (4 x 65 = 260)
IM_U, IM_V = 22, 69            # img chunk slots: rows -4..17, cols -4..64
KT = NT * QW                   # 1170 map elems per (tap, quarter)
PRE_P = G * 3 * NQ             # 72 precompute partitions (g, rgb, q)
CPQ = 390                      # precompute PSUM chunk (3 x 390 = 1170)

# spatial gaussian (5x5, sigma=5), normalized
_ax = np.arange(5, dtype=np.float64) - 2
_xx, _yy = np.meshgrid(_ax, _ax, indexing="ij")
_g = np.exp(-(_xx**2 + _yy**2) / (2 * 5.0**2))
SW = (_g / _g.sum()).astype(np.float64)
WC = float(SW[2, 2])           # center weight (spatial only; color=1)
# 12 unique taps (positive half-window); opposite taps share weight maps.
# dy=0 taps first: their muls (and mirrors) don't read halo rows, covering
# the intra-iteration halo-refresh DMA latency.
TAPS = [(0, 1), (0, 2), (1, -2), (1, -1), (1, 0), (1, 1), (1, 2),
        (2, -2), (2, -1), (2, 0), (2, 1), (2, 2)]

_BASS_CACHE = {}


def _build_bass():
    import concourse.bass as bass
    import concourse.mybir as mybir
    from concourse import tile

    f32 = mybir.dt.float32
    f32r = mybir.dt.float32r
    f16 = mybir.dt.float16
    AF = mybir.ActivationFunctionType
    OP = mybir.AluOpType

    nc = bass.Bass("TRN2", target_bir_lowering=False, debug=False,
                   enable_asserts=False)

    lg_d = nc.dram_tensor("lg", [P, F], f32, kind="ExternalInput")
    img_d = nc.dram_tensor("img", [PRE_P, IM_U * IM_V], f32,
                           kind="ExternalInput")
    mneg_d = nc.dram_tensor("mneg", [P, P], f16, kind="ExternalInput")
    mneg2_d = nc.dram_tensor("mneg2", [P, P], f16, kind="ExternalInput")
    iden_d = nc.dram_tensor("iden", [P, P], f16, kind="ExternalInput")
    onesd_d = nc.dram_tensor("onesd", [P, G], f16, kind="ExternalInput")
    bneg_d = nc.dram_tensor("bneg", [G, P], f16, kind="ExternalInput")
    rmask_d = nc.dram_tensor("rmask", [PRE_P, G * NQ], f16,
                             kind="ExternalInput")
    lns_d = nc.dram_tensor("lns", [G * NQ, 12], f32, kind="ExternalInput")
    qout_d = nc.dram_tensor("qout", [P, F], f32, kind="ExternalOutput")

    with tile.TileContext(nc) as tc, ExitStack() as ctx:
        const_pool = ctx.enter_context(tc.tile_pool(name="const", bufs=1))
        main_pool = ctx.enter_context(tc.tile_pool(name="main", bufs=1))

        mneg_t = const_pool.tile([P, P], f16, tag="mneg")
        nc.sync.dma_start(mneg_t[:], mneg_d.ap())
        mneg2_t = const_pool.tile([P, P], f16, tag="mneg2")
        nc.sync.dma_start(mneg2_t[:], mneg2_d.ap())
        iden_t = const_pool.tile([P, P], f16, tag="iden")
        nc.sync.dma_start(iden_t[:], iden_d.ap())
        onesd_t = const_pool.tile([P, G], f16, tag="onesd")
        nc.sync.dma_start(onesd_t[:], onesd_d.ap())
        bneg_t = const_pool.tile([G, P], f16, tag="bneg")
        nc.sync.dma_start(bneg_t[:], bneg_d.ap())
        rmask_t = const_pool.tile([PRE_P, G * NQ], f16, tag="rmask")
        nc.sync.dma_start(rmask_t[:], rmask_d.ap())
        lns_t = const_pool.tile([G * NQ, 12], f32, tag="lns")
        nc.sync.dma_start(lns_t[:], lns_d.ap())

        lg_t = main_pool.tile([P, F], f32, tag="lg")
        nc.sync.dma_start(lg_t[:], lg_d.ap())
        lg2_t = main_pool.tile([P, F], f16, tag="lg2")
        nc.scalar.copy(lg2_t[:], lg_t[:])  # fp16 PE-feed copy of logits

        # Absorber matmuls: pre-observe every stationary's DMA queue with a
        # tiny dummy matmul so real matmuls carry at most 1 extra wait.
        with tc.tile_pool(name="scrp", bufs=1, space="PSUM") as scrp:
            scr = scrp.tile([G, 2], f32, tag="scr")
            nc.tensor.matmul(scr[:1, :], mneg_t[:, 0:1], mneg_t[:, 0:2],
                             start=True, stop=True)
            nc.tensor.matmul(scr[:1, :], mneg2_t[:, 0:1], mneg2_t[:, 0:2],
                             start=True, stop=True)
            nc.tensor.matmul(scr[:1, :], iden_t[:, 0:1], iden_t[:, 0:2],
                             start=True, stop=True)
            nc.tensor.matmul(scr[:, :], onesd_t[:], onesd_t[:, 0:2],
                             start=True, stop=True)
            nc.tensor.matmul(scr[:1, :], bneg_t[:, 0:1], bneg_t[:, 0:2],
                             start=True, stop=True)
            nc.tensor.matmul(scr[:1, :], rmask_t[:, 0:1], rmask_t[:, 0:2],
                             start=True, stop=True)

        q_t = main_pool.tile([P, NT * NV], f16, tag="q")
        nc.vector.memset(q_t[:], 0.0)
        q3 = q_t[:].rearrange("p (t v) -> p t v", v=NV)

        w_all = main_pool.tile([P, 12 * NT * NV], f16, tag="wall")

        # ---------------- w-map precompute (column-sharded) ----------------
        with tc.tile_pool(name="pre", bufs=1) as prep, \
             tc.tile_pool(name="pre2", bufs=3) as prep2, \
             tc.tile_pool(name="psp", bufs=2, space="PSUM") as psp:
            img_t = prep.tile([PRE_P, IM_U * IM_V], f32, tag="img")
            nc.sync.dma_start(img_t[:], img_d.ap())
            img3 = img_t[:].rearrange("p (u v) -> p u v", v=IM_V)
            compact_t = prep.tile([PRE_P // 3, 12 * KT], f16, tag="compact")

            for ki, (dy, dx) in enumerate(TAPS):
                diff_t = prep2.tile([PRE_P, KT], f16, tag="diff")
                diff3 = diff_t[:].rearrange("p (t v) -> p t v", v=QW)
                nc.vector.tensor_sub(
                    diff3[:, :, :],
                    img3[:, 2 + dy:2 + dy + NT, 2 + dx:2 + dx + QW],
                    img3[:, 2:2 + NT, 2:2 + QW],
                )
                sq_t = prep2.tile([PRE_P, KT], f16, tag="sq")
                nc.vector.tensor_mul(sq_t[:], diff_t[:], diff_t[:])
                for cc in range(3):
                    sl = slice(cc * CPQ, (cc + 1) * CPQ)
                    d2_ps = psp.tile([G * NQ, CPQ], f32, tag="d2")
                    nc.tensor.matmul(d2_ps[:], rmask_t[:], sq_t[:, sl],
                                     start=True, stop=True)
                    nc.scalar.activation(
                        compact_t[:, ki * KT + cc * CPQ:
                                  ki * KT + (cc + 1) * CPQ],
                        d2_ps[:], AF.Exp, scale=-50.0,
                        bias=lns_t[:, ki:ki + 1])

            # broadcast: compact [(g,q), 12*1170] -> w_all [(g,c), 12*4680]
            comp_g = compact_t[:].rearrange("(g q) f -> g q f", q=NQ)
            wall_g = w_all[:].rearrange(
                "(g c) (k t v) -> g c k t v", c=C, t=NT, v=NV)
            for ch in range(C):
                for qt in range(NQ):
                    nc.sync.dma_start(
                        wall_g[:, ch, :, :, qt * QW:(qt + 1) * QW],
                        comp_g[:, qt, :].rearrange(
                            "g (k t v) -> g k t v", t=NT, v=QW))

        # ---------------- iteration machinery ----------------
        z_pool = ctx.enter_context(tc.tile_pool(name="zps", bufs=1,
                                                space="PSUM"))
        d_pool = ctx.enter_context(tc.tile_pool(name="dps", bufs=1,
                                                space="PSUM"))
        t_pool = ctx.enter_context(tc.tile_pool(name="tprod", bufs=3))
        e_pool = ctx.enter_context(tc.tile_pool(name="E", bufs=2))
        ln_pool = ctx.enter_context(tc.tile_pool(name="ln", bufs=2))

        def z_banks():
            return [z_pool.tile([P, CH], f32, tag=f"z{c}", name=f"z{c}")
                    for c in range(NCH)]

        def tail(zs, last):
            """exp -> D-reduce -> ln -> -lnD broadcast -> exp(q)."""
            lg3 = lg_t[:].rearrange("p (r x) -> p r x", x=W)
            for c in range(NCH):
                e_t = e_pool.tile([P, CH], f16, tag="E")
                nc.scalar.activation(e_t[:], zs[c][:], AF.Exp)
                d_ps = d_pool.tile([G, CH], f32, tag="D")
                nc.tensor.matmul(d_ps[:], onesd_t[:], e_t[:],
                                 start=True, stop=True)
                ln_t = ln_pool.tile([G, CH], f16, tag="ln")
                nc.scalar.activation(ln_t[:], d_ps[:], AF.Ln)
                nc.tensor.matmul(zs[c][:], bneg_t[:], ln_t[:],
                                 start=False, stop=True,
                                 skip_group_check=True)
                if last:
                    nc.scalar.activation(lg3[:, 2 * c:2 * c + 2, 0:W],
                                         zs[c][:], AF.Exp)
                else:
                    nc.scalar.activation(
                        q3[:, 2 + 2 * c:4 + 2 * c, 2:2 + W], zs[c][:], AF.Exp)

        # init: q0 = softmax(logits)
        zs = z_banks()
        for c in range(NCH):
            nc.tensor.matmul(zs[c][:], iden_t[:], lg2_t[:, c * CH:(c + 1) * CH],
                             start=True, stop=False, skip_group_check=True)
        tail(zs, last=False)

        for it in range(NUM_ITERS):
            last = it == NUM_ITERS - 1
            # refresh intra-core group halos (2 SBUF->SBUF DMAs)
            nc.sync.dma_start(q3[21:126, 0:2, 0:NV], q3[0:105, 14:16, 0:NV])
            nc.sync.dma_start(q3[0:105, 16:18, 0:NV], q3[21:126, 2:4, 0:NV])

            zs = z_banks()
            for c in range(NCH):
                sl = slice(c * CH, (c + 1) * CH)
                nc.tensor.matmul(zs[c][:], iden_t[:], lg2_t[:, sl],
                                 start=True, stop=False,
                                 skip_group_check=True)
                nc.tensor.matmul(zs[c][:], mneg2_t[:],
                                 q3[:, 2 + 2 * c:4 + 2 * c, 2:2 + W],
                                 start=False, stop=False,
                                 skip_group_check=True)

            # bilateral: 24 taps = 12 maps x {gather, scatter-sym}; products
            # on DVE, accumulation via mneg matmuls into the z banks.
            for ki, (dy, dx) in enumerate(TAPS):
                w3 = w_all[:, ki * NT * NV:(ki + 1) * NT * NV].rearrange(
                    "p (t v) -> p t v", v=NV)
                for (qdy, qdx, wdy, wdx) in ((dy, dx, 0, 0),
                                             (-dy, -dx, -dy, -dx)):
                    q_ap = q3[:, 2 + qdy:2 + qdy + RG, 2 + qdx:2 + qdx + W]
                    w_ap = w3[:, 2 + wdy:2 + wdy + RG, 2 + wdx:2 + wdx + W]
                    t_t = t_pool.tile([P, F], f16, tag="t", name="t_t")
                    t3 = t_t[:].rearrange("p (r x) -> p r x", x=W)
                    nc.vector.tensor_mul(t3[:, 0:RG, 0:W], q_ap, w_ap)
                    for c in range(NCH):
                        nc.tensor.matmul(
                            zs[c][:], mneg_t[:], t_t[:, c * CH:(c + 1) * CH],
                            start=False, stop=False, skip_group_check=True)

            tail(zs, last=last)

        nc.sync.dma_start(qout_d.ap(), lg_t[:])

    _legalize_matmul_waits(nc, mybir)
    return nc


def _legalize_matmul_waits(nc, mybir, max_waits=2):
    """TRN2 ISA sync-wait structs hold few waits per instruction; codegen
    aborts on more. Move excess waits onto InstNoOps (1 wait each) inserted
    right before on the same engine."""
    cap = {}
    for f in nc.m.functions:
        for blk in f.blocks:
            insts = blk.instructions
            out = []
            changed = False
            for i in insts:
                si = getattr(i, "sync_info", None)
                eng = getattr(i, "engine", None)
                max_waits = cap.get(type(i).__name__, 1)
                if (si is not None and eng is not None
                        and len(si.on_wait) > max_waits):
                    waits = list(si.on_wait)
                    keep, move = [], []
                    for w in waits:
                        if "PE" in w.ant_name and len(keep) < max_waits:
                            keep.append(w)
                        else:
                            move.append(w)
                    while len(keep) < max_waits and move:
                        keep.append(move.pop())
                    nop_cap = cap.get("InstNoOp", 1)
                    while move:
                        grp, move = move[:nop_cap], move[nop_cap:]
                        nop = mybir.InstNoOp(
                            name=nc.get_next_instruction_name(),
                            engine=eng, ins=[], outs=[])
                        nop.sync_info = mybir.SyncInfo(on_wait=grp,
                                                       on_update=[])
                        out.append(nop)
                    i.sync_info = mybir.SyncInfo(
                        on_wait=keep, on_update=list(si.on_update))
                    changed = True
                out.append(i)
            if changed:
                blk.instructions = out


def _prep_shards(logits, img, compat):
    """Host-side shard prep -> list of 8 in_maps."""
    mneg = np.kron(np.eye(G), -compat.T.astype(np.float64)).astype(np.float16)
    mneg2 = ((1.0 + WC) * np.kron(np.eye(G), -compat.T.astype(np.float64))
             ).astype(np.float16)
    iden = np.eye(P, dtype=np.float16)
    onesd = np.kron(np.eye(G), np.ones((C, 1))).astype(np.float16)
    bneg = np.kron(np.eye(G), -np.ones((1, C))).astype(np.float16)
    # rmask [(g,rgb,q), (g,q)]: sums rgb
    rmask = np.zeros((PRE_P, G * NQ), np.float16)
    for g in range(G):
        for rgb in range(3):
            for qt in range(NQ):
                rmask[(g * 3 + rgb) * NQ + qt, g * NQ + qt] = 1.0
    lns = np.tile(
        np.array([math.log(SW[2 + dy, 2 + dx]) for (dy, dx) in TAPS],
                 np.float32)[None, :], (G * NQ, 1))

    in_maps = []
    for core in range(8):
        b, j = divmod(core, 4)
        s = STARTS[j]
        lg = logits[b, :, s:s + 84, :].reshape(C, G, RG, W)
        lg = np.ascontiguousarray(
            lg.transpose(1, 0, 2, 3).reshape(P, F)).astype(np.float32)
        im = np.zeros((G, 3, NQ, IM_U, IM_V), np.float32)
        for g in range(G):
            rbase = s + g * RG - 4
            u0, u1 = max(0, -rbase), min(IM_U, H - rbase)
            for qt in range(NQ):
                cbase = qt * QW - 4
                v0, v1 = max(0, -cbase), min(IM_V, W - cbase)
                im[g, :, qt, u0:u1, v0:v1] = img[
                    b, :, rbase + u0:rbase + u1, cbase + v0:cbase + v1]
        im = im.reshape(PRE_P, IM_U * IM_V)
        in_maps.append({
            "lg": lg, "img": np.ascontiguousarray(im),
            "mneg": mneg, "mneg2": mneg2, "iden": iden, "onesd": onesd,
            "bneg": bneg, "rmask": rmask, "lns": lns,
        })
    return in_maps


def kernel(**inputs):
    logits = np.asarray(inputs["logits"], dtype=np.float32)
    img = np.asarray(inputs["img"], dtype=np.float32)
    compat = np.asarray(inputs["compat_mat"], dtype=np.float32)

    from concourse.bass_utils import run_bass_kernel_spmd

    if "nc" not in _BASS_CACHE:
        _BASS_CACHE["nc"] = _build_bass()
    nc = _BASS_CACHE["nc"]

    in_maps = _prep_shards(logits, img, compat)
    res = run_bass_kernel_spmd(nc, in_maps, core_ids=list(range(8)))
    _BASS_CACHE["last_result"] = res

    out = np.zeros((B, C, H, W), np.float32)
    for core in range(8):
        b, j = divmod(core, 4)
        s = STARTS[j]
        lo, hi = OWN[j]
        qc = res.results[core]["qout"].reshape(G, C, RG, W)
        qc = qc.transpose(1, 0, 2, 3).reshape(C, 84, W)
        out[b, :, s + lo:s + hi, :] = qc[:, lo:hi, :]
    return out


# revision 10
# speedup vs baseline: 1.9513x; 1.3695x over previous
"""CRF-as-RNN mean-field kernel for Trainium2 (Bass/Tile), 8-core SPMD.

Strategy (v2):
- Shard 2 images x 4 row-strips across 8 cores; 84 rows/core (64 owned +
  halo), 5 mean-field iterations shrink validity, no inter-core comms.
- Partitions = 6 row-groups x 21 channels = 126; free = 14 rows x 256 cols
  (+2 halos: 18 x 260 slots).
- The 5x5 spatial gaussian (sigma=0.1) is a delta => sp == q, folded into
  a second stationary mneg2 = (1+wc) * mneg applied to q via PE.
- Bilateral: 24 shifted products on DVE (fp16 2x), accumulated ON THE PE
  via mneg x t_k matmuls into 7 persistent PSUM z-banks (compat transform
  is linear). No DVE adds, no SBUF acc tile, f32 accumulation.
- Softmax: z-banks already hold logits + pairwise (logits fed as f32
  bitcast to f32r: full PE rate at 512 cols). exp/ln on ACT; lnD
  broadcast back via bneg (f32r) matmul; final exp writes q (fp16) or the
  f32 output tile.
- w-precompute, column-sharded: partitions (group, rgb, quarter) = 72;
  per tap: DVE diff, DVE square, PE rgb-reduce -> [24, 390] PSUM x3,
  ACT exp(-50*d2 + ln(spatial)) -> compact [24, 12*1170] fp16; then 84
  strided-partition DMAs replicate rows to the 21 channel partitions
  (w_all [126, 12*4680] fp16).
"""

import math
import sys
from contextlib import ExitStack

import numpy as np

sys.path.insert(0, "/opt/trn_rl_repo")

# ---------------- problem constants (hardcoded per contract) ----------------
B, C, H, W = 2, 21, 256, 256
G, RG = 6, 14                  # row groups per strip, rows per group
P = G * C                      # 126 partitions
F = RG * W                     # 3584 free elems per partition
NT, NV = 18, 260               # q/w map slots: rows -2..15, cols -2..257
STARTS = [0, 54, 118, 172]     # strip start rows
OWN = [(0, 64), (10, 74), (10, 74), (20, 84)]  # owned local-row range
NUM_ITERS = 5
NCH, CH = 7, 512               # softmax chunks (512 px = 2 rows)
NQ, XW, SS = 4, 64, 72         # col chunks: 4 x 64 owned px, 72 stored slots
IM_U, IM_V = 22, 77            # img chunk slots: rows -4..17, 77 cols
KT = NT * SS                   # 1296 map elems per (tap, chunk)
WT = NQ * KT                   # 5184 w elems per tap
PRE_P = G * 3 * NQ             # 72 precompute partitions (g, rgb, chunk)
CPQ = 432                      # precompute PSUM chunk (3 x 432 = 1296)

# spatial gaussian (5x5, sigma=5), normalized
_ax = np.arange(5, dtype=np.float64) - 2
_xx, _yy = np.meshgrid(_ax, _ax, indexing="ij")
_g = np.exp(-(_xx**2 + _yy**2) / (2 * 5.0**2))
SW = (_g / _g.sum()).astype(np.float64)
WC = float(SW[2, 2])           # center weight (spatial only; color=1)
# 12 unique taps (positive half-window); opposite taps share weight maps.
# dy=0 taps first: their muls (and mirrors) don't read halo rows, covering
# the intra-iteration halo-refresh DMA latency.
TAPS = [(0, 1), (0, 2), (1, -2), (1, -1), (1, 0), (1, 1), (1, 2),
        (2, -2), (2, -1), (2, 0), (2, 1), (2, 2)]

_BASS_CACHE = {}


def _build_bass():
    import concourse.bass as bass
    import concourse.mybir as mybir
    from concourse import tile

    f32 = mybir.dt.float32
    f32r = mybir.dt.float32r
    f16 = mybir.dt.float16
    AF = mybir.ActivationFunctionType
    OP = mybir.AluOpType

    nc = bass.Bass("TRN2", target_bir_lowering=False, debug=False,
                   enable_asserts=False)

    lg_d = nc.dram_tensor("lg", [P, F], f32, kind="ExternalInput")
    img_d = nc.dram_tensor("img", [PRE_P, IM_U * IM_V], f32,
                           kind="ExternalInput")
    mneg_d = nc.dram_tensor("mneg", [P, P], f16, kind="ExternalInput")
    mneg2_d = nc.dram_tensor("mneg2", [P, P], f16, kind="ExternalInput")
    iden_d = nc.dram_tensor("iden", [P, P], f16, kind="ExternalInput")
    onesd_d = nc.dram_tensor("onesd", [P, G], f16, kind="ExternalInput")
    bneg_d = nc.dram_tensor("bneg", [G, P], f16, kind="ExternalInput")
    rmask_d = nc.dram_tensor("rmask", [PRE_P, G * NQ], f16,
                             kind="ExternalInput")
    lns_d = nc.dram_tensor("lns", [G * NQ, 12], f32, kind="ExternalInput")
    qout_d = nc.dram_tensor("qout", [P, F], f32, kind="ExternalOutput")

    with tile.TileContext(nc) as tc, ExitStack() as ctx:
        const_pool = ctx.enter_context(tc.tile_pool(name="const", bufs=1))
        main_pool = ctx.enter_context(tc.tile_pool(name="main", bufs=1))

        mneg_t = const_pool.tile([P, P], f16, tag="mneg")
        nc.sync.dma_start(mneg_t[:], mneg_d.ap())
        mneg2_t = const_pool.tile([P, P], f16, tag="mneg2")
        nc.sync.dma_start(mneg2_t[:], mneg2_d.ap())
        iden_t = const_pool.tile([P, P], f16, tag="iden")
        nc.sync.dma_start(iden_t[:], iden_d.ap())
        onesd_t = const_pool.tile([P, G], f16, tag="onesd")
        nc.sync.dma_start(onesd_t[:], onesd_d.ap())
        bneg_t = const_pool.tile([G, P], f16, tag="bneg")
        nc.sync.dma_start(bneg_t[:], bneg_d.ap())
        rmask_t = const_pool.tile([PRE_P, G * NQ], f16, tag="rmask")
        nc.sync.dma_start(rmask_t[:], rmask_d.ap())
        lns_t = const_pool.tile([G * NQ, 12], f32, tag="lns")
        nc.sync.dma_start(lns_t[:], lns_d.ap())

        lg_t = main_pool.tile([P, F], f32, tag="lg")
        nc.sync.dma_start(lg_t[:], lg_d.ap())
        lg2_t = main_pool.tile([P, F], f16, tag="lg2")
        nc.scalar.copy(lg2_t[:], lg_t[:])  # fp16 PE-feed copy of logits

        # Absorber matmuls: pre-observe every stationary's DMA queue with a
        # tiny dummy matmul so real matmuls carry at most 1 extra wait.
        with tc.tile_pool(name="scrp", bufs=1, space="PSUM") as scrp:
            scr = scrp.tile([G, 2], f32, tag="scr")
            nc.tensor.matmul(scr[:1, :], mneg_t[:, 0:1], mneg_t[:, 0:2],
                             start=True, stop=True)
            nc.tensor.matmul(scr[:1, :], mneg2_t[:, 0:1], mneg2_t[:, 0:2],
                             start=True, stop=True)
            nc.tensor.matmul(scr[:1, :], iden_t[:, 0:1], iden_t[:, 0:2],
                             start=True, stop=True)
            nc.tensor.matmul(scr[:, :], onesd_t[:], onesd_t[:, 0:2],
                             start=True, stop=True)
            nc.tensor.matmul(scr[:1, :], bneg_t[:, 0:1], bneg_t[:, 0:2],
                             start=True, stop=True)
            nc.tensor.matmul(scr[:1, :], rmask_t[:, 0:1], rmask_t[:, 0:2],
                             start=True, stop=True)

        q_t = main_pool.tile([P, NT * NV], f16, tag="q")
        nc.vector.memset(q_t[:], 0.0)
        q3 = q_t[:].rearrange("p (t v) -> p t v", v=NV)

        w_all = main_pool.tile([P, 12 * WT], f16, tag="wall")

        # ---------------- w-map precompute (column-sharded) ----------------
        with tc.tile_pool(name="pre", bufs=1) as prep, \
             tc.tile_pool(name="pre2", bufs=3) as prep2, \
             tc.tile_pool(name="psp", bufs=2, space="PSUM") as psp:
            img_t = prep.tile([PRE_P, IM_U * IM_V], f32, tag="img")
            nc.sync.dma_start(img_t[:], img_d.ap())
            img3 = img_t[:].rearrange("p (u v) -> p u v", v=IM_V)
            compact_t = prep.tile([PRE_P // 3, 12 * KT], f16, tag="compact")

            for ki, (dy, dx) in enumerate(TAPS):
                diff_t = prep2.tile([PRE_P, KT], f16, tag="diff")
                diff3 = diff_t[:].rearrange("p (t v) -> p t v", v=SS)
                nc.vector.tensor_sub(
                    diff3[:, :, :],
                    img3[:, 2 + dy:2 + dy + NT, 2 + dx:2 + dx + SS],
                    img3[:, 2:2 + NT, 2:2 + SS],
                )
                sq_t = prep2.tile([PRE_P, KT], f16, tag="sq")
                nc.vector.tensor_mul(sq_t[:], diff_t[:], diff_t[:])
                for cc in range(3):
                    sl = slice(cc * CPQ, (cc + 1) * CPQ)
                    d2_ps = psp.tile([G * NQ, CPQ], f32, tag="d2")
                    nc.tensor.matmul(d2_ps[:], rmask_t[:], sq_t[:, sl],
                                     start=True, stop=True)
                    nc.scalar.activation(
                        compact_t[:, ki * KT + cc * CPQ:
                                  ki * KT + (cc + 1) * CPQ],
                        d2_ps[:], AF.Exp, scale=-50.0,
                        bias=lns_t[:, ki:ki + 1])

            # broadcast: compact [(g,q), 12*1296] -> w_all [(g,c), 12*4*1296]
            # per (ch, qt): 12 contiguous 2592-B runs per partition.
            comp_g = compact_t[:].rearrange("(g q) f -> g q f", q=NQ)
            wall_g = w_all[:].rearrange(
                "(g c) (k q f) -> g c k q f", c=C, q=NQ, f=KT)
            for ch in range(C):
                eng = nc.sync if ch % 2 == 0 else nc.scalar
                for qt in range(NQ):
                    eng.dma_start(
                        wall_g[:, ch, :, qt, :],
                        comp_g[:, qt, :].rearrange("g (k f) -> g k f", f=KT))

        # ---------------- iteration machinery ----------------
        z_pool = ctx.enter_context(tc.tile_pool(name="zps", bufs=1,
                                                space="PSUM"))
        d_pool = ctx.enter_context(tc.tile_pool(name="dps", bufs=1,
                                                space="PSUM"))
        t_pool = ctx.enter_context(tc.tile_pool(name="tprod", bufs=3))
        e_pool = ctx.enter_context(tc.tile_pool(name="E", bufs=2))
        ln_pool = ctx.enter_context(tc.tile_pool(name="ln", bufs=2))

        def z_banks():
            return [z_pool.tile([P, CH], f32, tag=f"z{c}", name=f"z{c}")
                    for c in range(NCH)]

        def tail(zs, last):
            """exp -> D-reduce -> ln -> -lnD broadcast -> exp(q)."""
            lg3 = lg_t[:].rearrange("p (r x) -> p r x", x=W)
            for c in range(NCH):
                e_t = e_pool.tile([P, CH], f16, tag="E")
                nc.scalar.activation(e_t[:], zs[c][:], AF.Exp)
                d_ps = d_pool.tile([G, CH], f32, tag="D")
                nc.tensor.matmul(d_ps[:], onesd_t[:], e_t[:],
                                 start=True, stop=True)
                ln_t = ln_pool.tile([G, CH], f16, tag="ln")
                nc.scalar.activation(ln_t[:], d_ps[:], AF.Ln)
                nc.tensor.matmul(zs[c][:], bneg_t[:], ln_t[:],
                                 start=False, stop=True,
                                 skip_group_check=True)
                if last:
                    nc.scalar.activation(lg3[:, 2 * c:2 * c + 2, 0:W],
                                         zs[c][:], AF.Exp)
                else:
                    nc.scalar.activation(
                        q3[:, 2 + 2 * c:4 + 2 * c, 2:2 + W], zs[c][:], AF.Exp)

        # init: q0 = softmax(logits)
        zs = z_banks()
        for c in range(NCH):
            nc.tensor.matmul(zs[c][:], iden_t[:], lg2_t[:, c * CH:(c + 1) * CH],
                             start=True, stop=False, skip_group_check=True)
        tail(zs, last=False)

        for it in range(NUM_ITERS):
            last = it == NUM_ITERS - 1
            # refresh intra-core group halos (2 SBUF->SBUF DMAs)
            nc.sync.dma_start(q3[21:126, 0:2, 0:NV], q3[0:105, 14:16, 0:NV])
            nc.sync.dma_start(q3[0:105, 16:18, 0:NV], q3[21:126, 2:4, 0:NV])

            zs = z_banks()
            for c in range(NCH):
                sl = slice(c * CH, (c + 1) * CH)
                nc.tensor.matmul(zs[c][:], iden_t[:], lg2_t[:, sl],
                                 start=True, stop=False,
                                 skip_group_check=True)
                nc.tensor.matmul(zs[c][:], mneg2_t[:],
                                 q3[:, 2 + 2 * c:4 + 2 * c, 2:2 + W],
                                 start=False, stop=False,
                                 skip_group_check=True)

            # bilateral: 24 taps = 12 maps x {gather, scatter-sym}; products
            # on DVE, accumulation via mneg matmuls into the z banks.
            for ki, (dy, dx) in enumerate(TAPS):
                w4 = w_all[:, ki * WT:(ki + 1) * WT].rearrange(
                    "p (q t v) -> p q t v", q=NQ, v=SS)
                for (qdy, qdx, wdy, wdx) in ((dy, dx, 0, 0),
                                             (-dy, -dx, -dy, -dx)):
                    q_ap = q3[:, 2 + qdy:2 + qdy + RG,
                              2 + qdx:2 + qdx + W].rearrange(
                        "p r (q x) -> p r q x", x=XW)
                    w_ap = w4[:, :, 2 + wdy:2 + wdy + RG,
                              2 + wdx:2 + wdx + XW].rearrange(
                        "p q r x -> p r q x")
                    t_t = t_pool.tile([P, F], f16, tag="t", name="t_t")
                    t4 = t_t[:].rearrange("p (r q x) -> p r q x", q=NQ, x=XW)
                    nc.vector.tensor_mul(t4[:, 0:RG, :, :], q_ap, w_ap)
                    for c in range(NCH):
                        nc.tensor.matmul(
                            zs[c][:], mneg_t[:], t_t[:, c * CH:(c + 1) * CH],
                            start=False, stop=False, skip_group_check=True)

            tail(zs, last=last)

        nc.sync.dma_start(qout_d.ap(), lg_t[:])

    _legalize_matmul_waits(nc, mybir)
    return nc


def _legalize_matmul_waits(nc, mybir, max_waits=2):
    """TRN2 ISA sync-wait structs hold few waits per instruction; codegen
    aborts on more. Move excess waits onto InstNoOps (1 wait each) inserted
    right before on the same engine."""
    cap = {}
    for f in nc.m.functions:
        for blk in f.blocks:
            insts = blk.instructions
            out = []
            changed = False
            for i in insts:
                si = getattr(i, "sync_info", None)
                eng = getattr(i, "engine", None)
                max_waits = cap.get(type(i).__name__, 1)
                if (si is not None and eng is not None
                        and len(si.on_wait) > max_waits):
                    waits = list(si.on_wait)
                    keep, move = [], []
                    for w in waits:
                        if "PE" in w.ant_name and len(keep) < max_waits:
                            keep.append(w)
                        else:
                            move.append(w)
                    while len(keep) < max_waits and move:
                        keep.append(move.pop())
                    nop_cap = cap.get("InstNoOp", 1)
                    while move:
                        grp, move = move[:nop_cap], move[nop_cap:]
                        nop = mybir.InstNoOp(
                            name=nc.get_next_instruction_name(),
                            engine=eng, ins=[], outs=[])
                        nop.sync_info = mybir.SyncInfo(on_wait=grp,
                                                       on_update=[])
                        out.append(nop)
                    i.sync_info = mybir.SyncInfo(
                        on_wait=keep, on_update=list(si.on_update))
                    changed = True
                out.append(i)
            if changed:
                blk.instructions = out


def _prep_shards(logits, img, compat):
    """Host-side shard prep -> list of 8 in_maps."""
    mneg = np.kron(np.eye(G), -compat.T.astype(np.float64)).astype(np.float16)
    mneg2 = ((1.0 + WC) * np.kron(np.eye(G), -compat.T.astype(np.float64))
             ).astype(np.float16)
    iden = np.eye(P, dtype=np.float16)
    onesd = np.kron(np.eye(G), np.ones((C, 1))).astype(np.float16)
    bneg = np.kron(np.eye(G), -np.ones((1, C))).astype(np.float16)
    # rmask [(g,rgb,q), (g,q)]: sums rgb
    rmask = np.zeros((PRE_P, G * NQ), np.float16)
    for g in range(G):
        for rgb in range(3):
            for qt in range(NQ):
                rmask[(g * 3 + rgb) * NQ + qt, g * NQ + qt] = 1.0
    lns = np.tile(
        np.array([math.log(SW[2 + dy, 2 + dx]) for (dy, dx) in TAPS],
                 np.float32)[None, :], (G * NQ, 1))

    in_maps = []
    for core in range(8):
        b, j = divmod(core, 4)
        s = STARTS[j]
        lg = logits[b, :, s:s + 84, :].reshape(C, G, RG, W)
        lg = np.ascontiguousarray(
            lg.transpose(1, 0, 2, 3).reshape(P, F)).astype(np.float32)
        im = np.zeros((G, 3, NQ, IM_U, IM_V), np.float32)
        for g in range(G):
            rbase = s + g * RG - 4
            u0, u1 = max(0, -rbase), min(IM_U, H - rbase)
            for qt in range(NQ):
                cbase = qt * XW - 4
                v0, v1 = max(0, -cbase), min(IM_V, W - cbase)
                im[g, :, qt, u0:u1, v0:v1] = img[
                    b, :, rbase + u0:rbase + u1, cbase + v0:cbase + v1]
        im = im.reshape(PRE_P, IM_U * IM_V)
        in_maps.append({
            "lg": lg, "img": np.ascontiguousarray(im),
            "mneg": mneg, "mneg2": mneg2, "iden": iden, "onesd": onesd,
            "bneg": bneg, "rmask": rmask, "lns": lns,
        })
    return in_maps


def kernel(**inputs):
    logits = np.asarray(inputs["logits"], dtype=np.float32)
    img = np.asarray(inputs["img"], dtype=np.float32)
    compat = np.asarray(inputs["compat_mat"], dtype=np.float32)

    from concourse.bass_utils import run_bass_kernel_spmd

    if "nc" not in _BASS_CACHE:
        _BASS_CACHE["nc"] = _build_bass()
    nc = _BASS_CACHE["nc"]

    in_maps = _prep_shards(logits, img, compat)
    res = run_bass_kernel_spmd(nc, in_maps, core_ids=list(range(8)))
    _BASS_CACHE["last_result"] = res

    out = np.zeros((B, C, H, W), np.float32)
    for core in range(8):
        b, j = divmod(core, 4)
        s = STARTS[j]
        lo, hi = OWN[j]
        qc = res.results[core]["qout"].reshape(G, C, RG, W)
        qc = qc.transpose(1, 0, 2, 3).reshape(C, 84, W)
        out[b, :, s + lo:s + hi, :] = qc[:, lo:hi, :]
    return out


# revision 12
# speedup vs baseline: 2.2531x; 1.1546x over previous
"""CRF-as-RNN mean-field kernel for Trainium2 (Bass/Tile), 8-core SPMD.

Strategy (v2):
- Shard 2 images x 4 row-strips across 8 cores; 84 rows/core (64 owned +
  halo), 5 mean-field iterations shrink validity, no inter-core comms.
- Partitions = 6 row-groups x 21 channels = 126; free = 14 rows x 256 cols
  (+2 halos: 18 x 260 slots).
- The 5x5 spatial gaussian (sigma=0.1) is a delta => sp == q, folded into
  a second stationary mneg2 = (1+wc) * mneg applied to q via PE.
- Bilateral: 24 shifted products on DVE (fp16 2x), accumulated ON THE PE
  via mneg x t_k matmuls into 7 persistent PSUM z-banks (compat transform
  is linear). No DVE adds, no SBUF acc tile, f32 accumulation.
- Softmax: z-banks already hold logits + pairwise (logits fed as f32
  bitcast to f32r: full PE rate at 512 cols). exp/ln on ACT; lnD
  broadcast back via bneg (f32r) matmul; final exp writes q (fp16) or the
  f32 output tile.
- w-precompute, column-sharded: partitions (group, rgb, quarter) = 72;
  per tap: DVE diff, DVE square, PE rgb-reduce -> [24, 390] PSUM x3,
  ACT exp(-50*d2 + ln(spatial)) -> compact [24, 12*1170] fp16; then 84
  strided-partition DMAs replicate rows to the 21 channel partitions
  (w_all [126, 12*4680] fp16).
"""

import math
import sys
from contextlib import ExitStack

import numpy as np

sys.path.insert(0, "/opt/trn_rl_repo")

# ---------------- problem constants (hardcoded per contract) ----------------
B, C, H, W = 2, 21, 256, 256
G, RG = 6, 14                  # row groups per strip, rows per group
P = G * C                      # 126 partitions
F = RG * W                     # 3584 free elems per partition
NT, NV = 18, 260               # q/w map slots: rows -2..15, cols -2..257
STARTS = [0, 54, 118, 172]     # strip start rows
OWN = [(0, 64), (10, 74), (10, 74), (20, 84)]  # owned local-row range
NUM_ITERS = 5
NCH, CH = 7, 512               # softmax chunks (512 px = 2 rows)
NQ, XW, SS = 4, 64, 72         # col chunks: 4 x 64 owned px, 72 stored slots
IM_U, IM_V = 22, 77            # img chunk slots: rows -4..17, 77 cols
KT = NT * SS                   # 1296 map elems per (tap, chunk)
WT = NQ * KT                   # 5184 w elems per tap
PRE_P = G * 3 * NQ             # 72 precompute partitions (g, rgb, chunk)
CPQ = 432                      # precompute PSUM chunk (3 x 432 = 1296)

# spatial gaussian (5x5, sigma=5), normalized
_ax = np.arange(5, dtype=np.float64) - 2
_xx, _yy = np.meshgrid(_ax, _ax, indexing="ij")
_g = np.exp(-(_xx**2 + _yy**2) / (2 * 5.0**2))
SW = (_g / _g.sum()).astype(np.float64)
WC = float(SW[2, 2])           # center weight (spatial only; color=1)
# 12 unique taps (positive half-window); opposite taps share weight maps.
# dy=0 taps first: their muls (and mirrors) don't read halo rows, covering
# the intra-iteration halo-refresh DMA latency.
TAPS = [(0, 1), (0, 2), (1, -2), (1, -1), (1, 0), (1, 1), (1, 2),
        (2, -2), (2, -1), (2, 0), (2, 1), (2, 2)]

_BASS_CACHE = {}


def _build_bass():
    import concourse.bass as bass
    import concourse.mybir as mybir
    from concourse import tile

    f32 = mybir.dt.float32
    f32r = mybir.dt.float32r
    f16 = mybir.dt.float16
    AF = mybir.ActivationFunctionType
    OP = mybir.AluOpType

    nc = bass.Bass("TRN2", target_bir_lowering=False, debug=False,
                   enable_asserts=False)

    lg_d = nc.dram_tensor("lg", [P, F], f32, kind="ExternalInput")
    img_d = nc.dram_tensor("img", [PRE_P, IM_U * IM_V], f32,
                           kind="ExternalInput")
    mneg_d = nc.dram_tensor("mneg", [P, P], f16, kind="ExternalInput")
    mneg2_d = nc.dram_tensor("mneg2", [P, P], f16, kind="ExternalInput")
    iden_d = nc.dram_tensor("iden", [P, P], f16, kind="ExternalInput")
    onesd_d = nc.dram_tensor("onesd", [P, G], f16, kind="ExternalInput")
    bneg_d = nc.dram_tensor("bneg", [G, P], f16, kind="ExternalInput")
    rmask_d = nc.dram_tensor("rmask", [PRE_P, G * NQ], f16,
                             kind="ExternalInput")
    lns_d = nc.dram_tensor("lns", [G * NQ, 12], f32, kind="ExternalInput")
    wbounce_d = nc.dram_tensor("wbounce", [PRE_P // 3, 12 * KT], f16,
                               kind="Internal")
    qout_d = nc.dram_tensor("qout", [P, F], f32, kind="ExternalOutput")

    with tile.TileContext(nc) as tc, ExitStack() as ctx:
        const_pool = ctx.enter_context(tc.tile_pool(name="const", bufs=1))
        main_pool = ctx.enter_context(tc.tile_pool(name="main", bufs=1))

        mneg_t = const_pool.tile([P, P], f16, tag="mneg")
        nc.sync.dma_start(mneg_t[:], mneg_d.ap())
        mneg2_t = const_pool.tile([P, P], f16, tag="mneg2")
        nc.sync.dma_start(mneg2_t[:], mneg2_d.ap())
        iden_t = const_pool.tile([P, P], f16, tag="iden")
        nc.sync.dma_start(iden_t[:], iden_d.ap())
        onesd_t = const_pool.tile([P, G], f16, tag="onesd")
        nc.sync.dma_start(onesd_t[:], onesd_d.ap())
        bneg_t = const_pool.tile([G, P], f16, tag="bneg")
        nc.sync.dma_start(bneg_t[:], bneg_d.ap())
        rmask_t = const_pool.tile([PRE_P, G * NQ], f16, tag="rmask")
        nc.sync.dma_start(rmask_t[:], rmask_d.ap())
        lns_t = const_pool.tile([G * NQ, 12], f32, tag="lns")
        nc.sync.dma_start(lns_t[:], lns_d.ap())

        lg_t = main_pool.tile([P, F], f32, tag="lg")
        nc.sync.dma_start(lg_t[:], lg_d.ap())
        lg2_t = main_pool.tile([P, F], f16, tag="lg2")
        nc.scalar.copy(lg2_t[:], lg_t[:])  # fp16 PE-feed copy of logits

        # Absorber matmuls: pre-observe every stationary's DMA queue with a
        # tiny dummy matmul so real matmuls carry at most 1 extra wait.
        with tc.tile_pool(name="scrp", bufs=1, space="PSUM") as scrp:
            scr = scrp.tile([G, 2], f32, tag="scr")
            nc.tensor.matmul(scr[:1, :], mneg_t[:, 0:1], mneg_t[:, 0:2],
                             start=True, stop=True)
            nc.tensor.matmul(scr[:1, :], mneg2_t[:, 0:1], mneg2_t[:, 0:2],
                             start=True, stop=True)
            nc.tensor.matmul(scr[:1, :], iden_t[:, 0:1], iden_t[:, 0:2],
                             start=True, stop=True)
            nc.tensor.matmul(scr[:, :], onesd_t[:], onesd_t[:, 0:2],
                             start=True, stop=True)
            nc.tensor.matmul(scr[:1, :], bneg_t[:, 0:1], bneg_t[:, 0:2],
                             start=True, stop=True)
            nc.tensor.matmul(scr[:1, :], rmask_t[:, 0:1], rmask_t[:, 0:2],
                             start=True, stop=True)

        q_ta = main_pool.tile([P, NT * NV], f16, tag="qa")
        nc.vector.memset(q_ta[:], 0.0)
        q_tb = main_pool.tile([P, NT * NV], f16, tag="qb")
        nc.vector.memset(q_tb[:], 0.0)
        q3a = q_ta[:].rearrange("p (t v) -> p t v", v=NV)
        q3b = q_tb[:].rearrange("p (t v) -> p t v", v=NV)

        w_all = main_pool.tile([P, 12 * WT], f16, tag="wall")

        # ---------------- w-map precompute (column-sharded) ----------------
        with tc.tile_pool(name="pre", bufs=1) as prep, \
             tc.tile_pool(name="pre2", bufs=2) as prep2, \
             tc.tile_pool(name="psp", bufs=2, space="PSUM") as psp:
            img_t = prep.tile([PRE_P, IM_U * IM_V], f32, tag="img")
            nc.sync.dma_start(img_t[:], img_d.ap())
            img3 = img_t[:].rearrange("p (u v) -> p u v", v=IM_V)

            for ki, (dy, dx) in enumerate(TAPS):
                diff_t = prep2.tile([PRE_P, KT], f16, tag="diff")
                diff3 = diff_t[:].rearrange("p (t v) -> p t v", v=SS)
                nc.vector.tensor_sub(
                    diff3[:, :, :],
                    img3[:, 2 + dy:2 + dy + NT, 2 + dx:2 + dx + SS],
                    img3[:, 2:2 + NT, 2:2 + SS],
                )
                sq_t = prep2.tile([PRE_P, KT], f16, tag="sq")
                nc.vector.tensor_mul(sq_t[:], diff_t[:], diff_t[:])
                ctap_t = prep2.tile([PRE_P // 3, KT], f16, tag="ctap")
                for cc in range(3):
                    sl = slice(cc * CPQ, (cc + 1) * CPQ)
                    d2_ps = psp.tile([G * NQ, CPQ], f32, tag="d2")
                    nc.tensor.matmul(d2_ps[:], rmask_t[:], sq_t[:, sl],
                                     start=True, stop=True)
                    nc.scalar.activation(ctap_t[:, sl], d2_ps[:], AF.Exp,
                                         scale=-50.0,
                                         bias=lns_t[:, ki:ki + 1])
                nc.sync.dma_start(
                    wbounce_d.ap()[:, ki * KT:(ki + 1) * KT], ctap_t[:])

            # broadcast via DRAM bounce: SBUF->SBUF DMAs run on only 6
            # engines; HBM reads spread across ~15. compact went to DRAM
            # per tap above; per (ch, qt) a contiguous 31-KB read here.
            wb4 = wbounce_d.ap().rearrange("(g q) (k f) -> g q k f",
                                           q=NQ, f=KT)
            wall_g = w_all[:].rearrange(
                "(g c) (k q f) -> g c k q f", c=C, q=NQ, f=KT)
            for ch in range(C):
                eng = nc.sync if ch % 2 == 0 else nc.scalar
                for qt in range(NQ):
                    eng.dma_start(wall_g[:, ch, :, qt, :], wb4[:, qt, :, :])

        # ---------------- iteration machinery ----------------
        z_pool = ctx.enter_context(tc.tile_pool(name="zps", bufs=1,
                                                space="PSUM"))
        d_pool = ctx.enter_context(tc.tile_pool(name="dps", bufs=1,
                                                space="PSUM"))
        t_pool = ctx.enter_context(tc.tile_pool(name="tprod", bufs=3))
        e_pool = ctx.enter_context(tc.tile_pool(name="E", bufs=2))
        ln_pool = ctx.enter_context(tc.tile_pool(name="ln", bufs=2))

        def z_banks():
            return [z_pool.tile([P, CH], f32, tag=f"z{c}", name=f"z{c}")
                    for c in range(NCH)]

        lg3 = lg_t[:].rearrange("p (r x) -> p r x", x=W)

        def tail(zs, chunks, q3n, last):
            """exp -> D-reduce -> ln -> -lnD broadcast -> exp(q)."""
            for c in chunks:
                e_t = e_pool.tile([P, CH], f16, tag="E")
                nc.scalar.activation(e_t[:], zs[c][:], AF.Exp)
                d_ps = d_pool.tile([G, CH], f32, tag="D")
                nc.tensor.matmul(d_ps[:], onesd_t[:], e_t[:],
                                 start=True, stop=True)
                ln_t = ln_pool.tile([G, CH], f16, tag="ln")
                nc.scalar.activation(ln_t[:], d_ps[:], AF.Ln)
                nc.tensor.matmul(zs[c][:], bneg_t[:], ln_t[:],
                                 start=False, stop=True,
                                 skip_group_check=True)
                if last:
                    nc.scalar.activation(lg3[:, 2 * c:2 * c + 2, 0:W],
                                         zs[c][:], AF.Exp)
                else:
                    nc.scalar.activation(
                        q3n[:, 2 + 2 * c:4 + 2 * c, 2:2 + W], zs[c][:],
                        AF.Exp)

        # init: q0 = softmax(logits) -> q_a
        zs = z_banks()
        for c in range(NCH):
            nc.tensor.matmul(zs[c][:], iden_t[:], lg2_t[:, c * CH:(c + 1) * CH],
                             start=True, stop=False, skip_group_check=True)
        tail(zs, range(NCH), q3a, last=False)

        # two row-halves per iteration: tails of half A hide under half B's
        # DVE/PE work; ping-pong q (read old, write new) makes that legal.
        HALVES = [(0, 8, range(0, 4)), (8, RG, range(4, NCH))]

        for it in range(NUM_ITERS):
            last = it == NUM_ITERS - 1
            q3o, q3n = (q3a, q3b) if it % 2 == 0 else (q3b, q3a)
            # refresh intra-core group halos of the OLD q
            nc.sync.dma_start(q3o[21:126, 0:2, 0:NV], q3o[0:105, 14:16, 0:NV])
            nc.sync.dma_start(q3o[0:105, 16:18, 0:NV], q3o[21:126, 2:4, 0:NV])

            zs = z_banks()
            for (r0, r1, chunks) in HALVES:
                nr = r1 - r0
                for c in chunks:
                    sl = slice(c * CH, (c + 1) * CH)
                    nc.tensor.matmul(zs[c][:], iden_t[:], lg2_t[:, sl],
                                     start=True, stop=False,
                                     skip_group_check=True)
                    nc.tensor.matmul(zs[c][:], mneg2_t[:],
                                     q3o[:, 2 + 2 * c:4 + 2 * c, 2:2 + W],
                                     start=False, stop=False,
                                     skip_group_check=True)
                for ki, (dy, dx) in enumerate(TAPS):
                    w4 = w_all[:, ki * WT:(ki + 1) * WT].rearrange(
                        "p (q t v) -> p q t v", q=NQ, v=SS)
                    for (qdy, qdx, wdy, wdx) in ((dy, dx, 0, 0),
                                                 (-dy, -dx, -dy, -dx)):
                        q_ap = q3o[:, 2 + qdy + r0:2 + qdy + r1,
                                   2 + qdx:2 + qdx + W].rearrange(
                            "p r (q x) -> p r q x", x=XW)
                        w_ap = w4[:, :, 2 + wdy + r0:2 + wdy + r1,
                                  2 + wdx:2 + wdx + XW].rearrange(
                            "p q r x -> p r q x")
                        t_t = t_pool.tile([P, nr * W], f16, tag=f"t{r0}",
                                          name="t_t")
                        t4 = t_t[:].rearrange("p (r q x) -> p r q x",
                                              q=NQ, x=XW)
                        nc.vector.tensor_mul(t4[:, :, :, :], q_ap, w_ap)
                        for c in chunks:
                            sl = slice((2 * c - r0) * W, (2 * c + 2 - r0) * W)
                            nc.tensor.matmul(
                                zs[c][:], mneg_t[:], t_t[:, sl],
                                start=False, stop=False,
                                skip_group_check=True)
                tail(zs, chunks, q3n, last=last)

        nc.sync.dma_start(qout_d.ap(), lg_t[:])

    _legalize_matmul_waits(nc, mybir)
    return nc


def _legalize_matmul_waits(nc, mybir, max_waits=2):
    """TRN2 ISA sync-wait structs hold few waits per instruction; codegen
    aborts on more. Move excess waits onto InstNoOps (1 wait each) inserted
    right before on the same engine."""
    cap = {}
    for f in nc.m.functions:
        for blk in f.blocks:
            insts = blk.instructions
            out = []
            changed = False
            for i in insts:
                si = getattr(i, "sync_info", None)
                eng = getattr(i, "engine", None)
                max_waits = cap.get(type(i).__name__, 1)
                if (si is not None and eng is not None
                        and len(si.on_wait) > max_waits):
                    waits = list(si.on_wait)
                    keep, move = [], []
                    for w in waits:
                        if "PE" in w.ant_name and len(keep) < max_waits:
                            keep.append(w)
                        else:
                            move.append(w)
                    while len(keep) < max_waits and move:
                        keep.append(move.pop())
                    nop_cap = cap.get("InstNoOp", 1)
                    while move:
                        grp, move = move[:nop_cap], move[nop_cap:]
                        nop = mybir.InstNoOp(
                            name=nc.get_next_instruction_name(),
                            engine=eng, ins=[], outs=[])
                        nop.sync_info = mybir.SyncInfo(on_wait=grp,
                                                       on_update=[])
                        out.append(nop)
                    i.sync_info = mybir.SyncInfo(
                        on_wait=keep, on_update=list(si.on_update))
                    changed = True
                out.append(i)
            if changed:
                blk.instructions = out


def _prep_shards(logits, img, compat):
    """Host-side shard prep -> list of 8 in_maps."""
    mneg = np.kron(np.eye(G), -compat.T.astype(np.float64)).astype(np.float16)
    mneg2 = ((1.0 + WC) * np.kron(np.eye(G), -compat.T.astype(np.float64))
             ).astype(np.float16)
    iden = np.eye(P, dtype=np.float16)
    onesd = np.kron(np.eye(G), np.ones((C, 1))).astype(np.float16)
    bneg = np.kron(np.eye(G), -np.ones((1, C))).astype(np.float16)
    # rmask [(g,rgb,q), (g,q)]: sums rgb
    rmask = np.zeros((PRE_P, G * NQ), np.float16)
    for g in range(G):
        for rgb in range(3):
            for qt in range(NQ):
                rmask[(g * 3 + rgb) * NQ + qt, g * NQ + qt] = 1.0
    lns = np.tile(
        np.array([math.log(SW[2 + dy, 2 + dx]) for (dy, dx) in TAPS],
                 np.float32)[None, :], (G * NQ, 1))

    in_maps = []
    for core in range(8):
        b, j = divmod(core, 4)
        s = STARTS[j]
        lg = logits[b, :, s:s + 84, :].reshape(C, G, RG, W)
        lg = np.ascontiguousarray(
            lg.transpose(1, 0, 2, 3).reshape(P, F)).astype(np.float32)
        im = np.zeros((G, 3, NQ, IM_U, IM_V), np.float32)
        for g in range(G):
            rbase = s + g * RG - 4
            u0, u1 = max(0, -rbase), min(IM_U, H - rbase)
            for qt in range(NQ):
                cbase = qt * XW - 4
                v0, v1 = max(0, -cbase), min(IM_V, W - cbase)
                im[g, :, qt, u0:u1, v0:v1] = img[
                    b, :, rbase + u0:rbase + u1, cbase + v0:cbase + v1]
        im = im.reshape(PRE_P, IM_U * IM_V)
        in_maps.append({
            "lg": lg, "img": np.ascontiguousarray(im),
            "mneg": mneg, "mneg2": mneg2, "iden": iden, "onesd": onesd,
            "bneg": bneg, "rmask": rmask, "lns": lns,
        })
    return in_maps


def kernel(**inputs):
    logits = np.asarray(inputs["logits"], dtype=np.float32)
    img = np.asarray(inputs["img"], dtype=np.float32)
    compat = np.asarray(inputs["compat_mat"], dtype=np.float32)

    from concourse.bass_utils import run_bass_kernel_spmd

    if "nc" not in _BASS_CACHE:
        _BASS_CACHE["nc"] = _build_bass()
    nc = _BASS_CACHE["nc"]

    in_maps = _prep_shards(logits, img, compat)
    res = run_bass_kernel_spmd(nc, in_maps, core_ids=list(range(8)))
    _BASS_CACHE["last_result"] = res

    out = np.zeros((B, C, H, W), np.float32)
    for core in range(8):
        b, j = divmod(core, 4)
        s = STARTS[j]
        lo, hi = OWN[j]
        qc = res.results[core]["qout"].reshape(G, C, RG, W)
        qc = qc.transpose(1, 0, 2, 3).reshape(C, 84, W)
        out[b, :, s + lo:s + hi, :] = qc[:, lo:hi, :]
    return out


# revision 19
# speedup vs baseline: 2.3682x; 1.0511x over previous
"""CRF-as-RNN mean-field kernel for Trainium2 (Bass/Tile), 8-core SPMD.

Strategy (v2):
- Shard 2 images x 4 row-strips across 8 cores; 84 rows/core (64 owned +
  halo), 5 mean-field iterations shrink validity, no inter-core comms.
- Partitions = 6 row-groups x 21 channels = 126; free = 14 rows x 256 cols
  (+2 halos: 18 x 260 slots).
- The 5x5 spatial gaussian (sigma=0.1) is a delta => sp == q, folded into
  a second stationary mneg2 = (1+wc) * mneg applied to q via PE.
- Bilateral: 24 shifted products on DVE (fp16 2x), accumulated ON THE PE
  via mneg x t_k matmuls into 7 persistent PSUM z-banks (compat transform
  is linear). No DVE adds, no SBUF acc tile, f32 accumulation.
- Softmax: z-banks already hold logits + pairwise (logits fed as f32
  bitcast to f32r: full PE rate at 512 cols). exp/ln on ACT; lnD
  broadcast back via bneg (f32r) matmul; final exp writes q (fp16) or the
  f32 output tile.
- w-precompute, column-sharded: partitions (group, rgb, quarter) = 72;
  per tap: DVE diff, DVE square, PE rgb-reduce -> [24, 390] PSUM x3,
  ACT exp(-50*d2 + ln(spatial)) -> compact [24, 12*1170] fp16; then 84
  strided-partition DMAs replicate rows to the 21 channel partitions
  (w_all [126, 12*4680] fp16).
"""

import math
import sys
from contextlib import ExitStack

import numpy as np

sys.path.insert(0, "/opt/trn_rl_repo")

# ---------------- problem constants (hardcoded per contract) ----------------
B, C, H, W = 2, 21, 256, 256
G, RG = 6, 14                  # row groups per strip, rows per group
P = G * C                      # 126 partitions
F = RG * W                     # 3584 free elems per partition
NT, NV = 18, 260               # q/w map slots: rows -2..15, cols -2..257
STARTS = [0, 54, 118, 172]     # strip start rows
OWN = [(0, 64), (10, 74), (10, 74), (20, 84)]  # owned local-row range
NUM_ITERS = 5
NCH, CH = 7, 512               # softmax chunks (512 px = 2 rows)
NQ, XW, SS = 4, 64, 72         # col chunks: 4 x 64 owned px, 72 stored slots
IM_U, IM_V = 22, 77            # img chunk slots: rows -4..17, 77 cols
KT = NT * SS                   # 1296 map elems per (tap, chunk)
WT = NQ * KT                   # 5184 w elems per tap
PRE_P = G * 3 * NQ             # 72 precompute partitions (g, rgb, chunk)
CPQ = 432                      # precompute PSUM chunk (3 x 432 = 1296)

# spatial gaussian (5x5, sigma=5), normalized
_ax = np.arange(5, dtype=np.float64) - 2
_xx, _yy = np.meshgrid(_ax, _ax, indexing="ij")
_g = np.exp(-(_xx**2 + _yy**2) / (2 * 5.0**2))
SW = (_g / _g.sum()).astype(np.float64)
WC = float(SW[2, 2])           # center weight (spatial only; color=1)
# 12 unique taps (positive half-window); opposite taps share weight maps.
# dy=0 taps first: their muls (and mirrors) don't read halo rows, covering
# the intra-iteration halo-refresh DMA latency.
TAPS = [(0, 1), (0, 2), (1, -2), (1, -1), (1, 0), (1, 1), (1, 2),
        (2, -2), (2, -1), (2, 0), (2, 1), (2, 2)]

_BASS_CACHE = {}


def _build_bass():
    import concourse.bass as bass
    import concourse.mybir as mybir
    from concourse import tile

    f32 = mybir.dt.float32
    f32r = mybir.dt.float32r
    f16 = mybir.dt.float16
    AF = mybir.ActivationFunctionType
    OP = mybir.AluOpType

    nc = bass.Bass("TRN2", target_bir_lowering=False, debug=False,
                   enable_asserts=False)

    lg_d = nc.dram_tensor("lg", [P, F], f32, kind="ExternalInput")
    img_d = nc.dram_tensor("img", [PRE_P, IM_U * IM_V], f32,
                           kind="ExternalInput")
    mneg_d = nc.dram_tensor("mneg", [P, P], f16, kind="ExternalInput")
    mneg2_d = nc.dram_tensor("mneg2", [P, P], f16, kind="ExternalInput")
    repl6_d = nc.dram_tensor("repl6", [44, 2 * P], f16,
                             kind="ExternalInput")
    onesd_d = nc.dram_tensor("onesd", [P, G], f16, kind="ExternalInput")
    bneg_d = nc.dram_tensor("bneg", [G, P], f16, kind="ExternalInput")
    rmask_d = nc.dram_tensor("rmask", [PRE_P, G * NQ], f16,
                             kind="ExternalInput")
    lns_d = nc.dram_tensor("lns", [44, 12], f32, kind="ExternalInput")
    wbounce_d = nc.dram_tensor("wbounce", [PRE_P // 3, 12 * KT], f16,
                               kind="Internal")
    qout_d = nc.dram_tensor("qout", [P, F], f32, kind="ExternalOutput")

    with tile.TileContext(nc) as tc, ExitStack() as ctx:
        const_pool = ctx.enter_context(tc.tile_pool(name="const", bufs=1))
        main_pool = ctx.enter_context(tc.tile_pool(name="main", bufs=1))

        mneg_t = const_pool.tile([P, P], f16, tag="mneg")
        nc.sync.dma_start(mneg_t[:], mneg_d.ap())
        mneg2_t = const_pool.tile([P, P], f16, tag="mneg2")
        nc.sync.dma_start(mneg2_t[:], mneg2_d.ap())
        repl6_t = const_pool.tile([44, 2 * P], f16, tag="repl6")
        nc.sync.dma_start(repl6_t[:], repl6_d.ap())
        onesd_t = const_pool.tile([P, G], f16, tag="onesd")
        nc.sync.dma_start(onesd_t[:], onesd_d.ap())
        bneg_t = const_pool.tile([G, P], f16, tag="bneg")
        nc.sync.dma_start(bneg_t[:], bneg_d.ap())
        rmask_t = const_pool.tile([PRE_P, G * NQ], f16, tag="rmask")
        nc.sync.dma_start(rmask_t[:], rmask_d.ap())
        lns_t = const_pool.tile([44, 12], f32, tag="lns")
        nc.sync.dma_start(lns_t[:], lns_d.ap())

        lg_t = main_pool.tile([P, F], f32, tag="lg")
        nc.sync.dma_start(lg_t[:], lg_d.ap())

        # Absorber matmuls: pre-observe every stationary's DMA queue with a
        # tiny dummy matmul so real matmuls carry at most 1 extra wait.
        with tc.tile_pool(name="scrp", bufs=1, space="PSUM") as scrp:
            scr = scrp.tile([G, 2], f32, tag="scr")
            nc.tensor.matmul(scr[:1, :], mneg_t[:, 0:1], mneg_t[:, 0:2],
                             start=True, stop=True)
            nc.tensor.matmul(scr[:1, :], mneg2_t[:, 0:1], mneg2_t[:, 0:2],
                             start=True, stop=True)
            nc.tensor.matmul(scr[:1, :], repl6_t[:, 0:1], repl6_t[:, 0:2],
                             start=True, stop=True)
            nc.tensor.matmul(scr[:, :], onesd_t[:], onesd_t[:, 0:2],
                             start=True, stop=True)
            nc.tensor.matmul(scr[:1, :], bneg_t[:, 0:1], bneg_t[:, 0:2],
                             start=True, stop=True)
            nc.tensor.matmul(scr[:1, :], rmask_t[:, 0:1], rmask_t[:, 0:2],
                             start=True, stop=True)

        q_ta = main_pool.tile([P, NT * NV], f16, tag="qa")
        nc.vector.memset(q_ta[:], 0.0)
        q_tb = main_pool.tile([P, NT * NV], f16, tag="qb")
        nc.vector.memset(q_tb[:], 0.0)
        q3a = q_ta[:].rearrange("p (t v) -> p t v", v=NV)
        q3b = q_tb[:].rearrange("p (t v) -> p t v", v=NV)

        w_all = main_pool.tile([P, 12 * WT], f16, tag="wall")
        e_pool = ctx.enter_context(tc.tile_pool(name="E", bufs=2))
        ln_pool = ctx.enter_context(tc.tile_pool(name="ln", bufs=2))

        # ---------------- w-map precompute (column-sharded) ----------------
        # d-bank is persistent (shared by init + iteration tails).
        d_pool = ctx.enter_context(tc.tile_pool(name="dps", bufs=1,
                                                space="PSUM"))
        DMA_TAPS = range(6, 12)    # replicated via DRAM-bounce DMA reads
        COMP_TAPS = range(0, 6)    # replicated via PE matmul + ACT/DVE copies
        with tc.tile_pool(name="pre", bufs=1) as prep, \
             tc.tile_pool(name="pre2", bufs=2) as prep2, \
             tc.tile_pool(name="psp", bufs=1, space="PSUM") as psp, \
             tc.tile_pool(name="bcp", bufs=2, space="PSUM") as bcp, \
             tc.tile_pool(name="izp", bufs=2, space="PSUM") as izp:
            img_t = prep.tile([PRE_P, IM_U * IM_V], f32, tag="img")
            nc.sync.dma_start(img_t[:], img_d.ap())
            img3 = img_t[:].rearrange("p (u v) -> p u v", v=IM_V)

            for ki, (dy, dx) in enumerate(TAPS):
                diff_t = prep2.tile([PRE_P, KT], f16, tag="diff")
                diff3 = diff_t[:].rearrange("p (t v) -> p t v", v=SS)
                nc.vector.tensor_sub(
                    diff3[:, :, :],
                    img3[:, 2 + dy:2 + dy + NT, 2 + dx:2 + dx + SS],
                    img3[:, 2:2 + NT, 2:2 + SS],
                )
                sq_t = prep2.tile([PRE_P, KT], f16, tag="sq")
                nc.vector.tensor_mul(sq_t[:], diff_t[:], diff_t[:])
                if ki in DMA_TAPS:
                    ctap_t = prep2.tile([PRE_P // 3, KT], f16, tag="ctap")
                    for cc in range(3):
                        sl = slice(cc * CPQ, (cc + 1) * CPQ)
                        d2_ps = psp.tile([G * NQ, CPQ], f32, tag="d2")
                        nc.tensor.matmul(d2_ps[:], rmask_t[:], sq_t[:, sl],
                                         start=True, stop=True)
                        nc.scalar.activation(ctap_t[:, sl], d2_ps[:], AF.Exp,
                                             scale=-50.0,
                                             bias=lns_t[0:24, ki:ki + 1])
                    nc.sync.dma_start(
                        wbounce_d.ap()[:, ki * KT:(ki + 1) * KT], ctap_t[:])
                else:
                    # PE replication [g -> (g,c)]: qt pairs staged at
                    # partition bases 0 and 32 (matmul moving-base rule);
                    # two masked stationaries pick rows 0-5 / 6-11.
                    c44_t = prep2.tile([44, KT], f16, tag="c44")
                    for cc in range(3):
                        sl = slice(cc * CPQ, (cc + 1) * CPQ)
                        d2a = psp.tile([2 * G, CPQ], f32, tag="d2a")
                        nc.tensor.matmul(d2a[:], rmask_t[:, 0:12],
                                         sq_t[:, sl], start=True, stop=True)
                        nc.scalar.activation(c44_t[0:12, sl], d2a[:],
                                             AF.Exp, scale=-50.0,
                                             bias=lns_t[0:12, ki:ki + 1])
                        d2b = psp.tile([2 * G, CPQ], f32, tag="d2b")
                        nc.tensor.matmul(d2b[:], rmask_t[:, 12:24],
                                         sq_t[:, sl], start=True, stop=True)
                        nc.scalar.activation(c44_t[32:44, sl], d2b[:],
                                             AF.Exp, scale=-50.0,
                                             bias=lns_t[32:44, ki:ki + 1])
                    for qt in range(NQ):
                        base = 0 if qt < 2 else 32
                        st = repl6_t[base:base + 12,
                                     (qt % 2) * P:(qt % 2 + 1) * P]
                        for cc, (o0, o1) in enumerate(
                                ((0, 512), (512, 1024), (1024, KT))):
                            b_ps = bcp.tile([P, 512], f32, tag="bc",
                                            name="b_ps")
                            nc.tensor.matmul(b_ps[:, 0:o1 - o0], st,
                                             c44_t[base:base + 12, o0:o1],
                                             start=True, stop=True)
                            wdst = w_all[:, (qt * 12 + ki) * KT + o0:
                                         (qt * 12 + ki) * KT + o1]
                            if (qt + cc) % 2 == 0:
                                nc.scalar.copy(wdst, b_ps[:, 0:o1 - o0])
                            else:
                                nc.vector.tensor_copy(wdst,
                                                      b_ps[:, 0:o1 - o0])

            # DMA-half broadcast via DRAM bounce, grouped (ch, tap-triple)
            # so each start is [6 parts, 4 qt-blocks of 7776 B].
            wb_flat = wbounce_d.ap().rearrange("r f -> (r f)")
            for ch in range(C):
                eng = nc.sync if ch % 2 == 0 else nc.scalar
                for tg in (2, 3):
                    src_ap = wbounce_d.ap().rearrange(
                        "(q g) (k f) -> g q k f", g=G, f=KT)[
                        :, :, tg * 3:(tg + 1) * 3, :]
                    wall_g = w_all[:].rearrange(
                        "(g c) (q k f) -> g c q k f", c=C, k=12, f=KT)
                    eng.dma_start(
                        wall_g[:, ch, :, tg * 3:(tg + 1) * 3, :], src_ap)

            # init: q0 = softmax(logits), rotating PSUM banks (z pool is
            # not allocated yet -- front pools own the banks).
            lg3i = lg_t[:].rearrange("p (r x) -> p r x", x=W)
            for c in range(NCH):
                iz = izp.tile([P, CH], f32, tag="iz", name="iz")
                nc.scalar.copy(iz[:], lg_t[:, c * CH:(c + 1) * CH])
                e_t = e_pool.tile([P, CH], f16, tag="E")
                nc.scalar.activation(e_t[:], iz[:], AF.Exp)
                d_ps = d_pool.tile([G, CH], f32, tag="D")
                nc.tensor.matmul(d_ps[:], onesd_t[:], e_t[:],
                                 start=True, stop=True)
                ln_t = ln_pool.tile([G, CH], f16, tag="ln")
                nc.scalar.activation(ln_t[:], d_ps[:], AF.Ln)
                nc.tensor.matmul(iz[:], bneg_t[:], ln_t[:],
                                 start=False, stop=True,
                                 skip_group_check=True)
                nc.scalar.activation(q3a[:, 2 + 2 * c:4 + 2 * c, 2:2 + W],
                                     iz[:], AF.Exp)

        # ---------------- iteration machinery ----------------
        z_pool = ctx.enter_context(tc.tile_pool(name="zps", bufs=1,
                                                space="PSUM"))
        t_pool = ctx.enter_context(tc.tile_pool(name="tprod", bufs=3))

        def z_banks():
            return [z_pool.tile([P, CH], f32, tag=f"z{c}", name=f"z{c}")
                    for c in range(NCH)]

        lg3 = lg_t[:].rearrange("p (r x) -> p r x", x=W)

        def tail(zs, chunks, q3n, last):
            """exp -> D-reduce -> ln -> -lnD broadcast -> exp(q)."""
            for c in chunks:
                e_t = e_pool.tile([P, CH], f16, tag="E")
                nc.scalar.activation(e_t[:], zs[c][:], AF.Exp)
                d_ps = d_pool.tile([G, CH], f32, tag="D")
                nc.tensor.matmul(d_ps[:], onesd_t[:], e_t[:],
                                 start=True, stop=True)
                ln_t = ln_pool.tile([G, CH], f16, tag="ln")
                nc.scalar.activation(ln_t[:], d_ps[:], AF.Ln)
                nc.tensor.matmul(zs[c][:], bneg_t[:], ln_t[:],
                                 start=False, stop=True,
                                 skip_group_check=True)
                if last:
                    nc.scalar.activation(lg3[:, 2 * c:2 * c + 2, 0:W],
                                         zs[c][:], AF.Exp)
                else:
                    nc.scalar.activation(
                        q3n[:, 2 + 2 * c:4 + 2 * c, 2:2 + W], zs[c][:],
                        AF.Exp)

        # two row-halves per iteration: tails of half A hide under half B's
        # DVE/PE work; ping-pong q (read old, write new) makes that legal.
        HALVES = [(0, 8, range(0, 4)), (8, RG, range(4, NCH))]

        for it in range(NUM_ITERS):
            last = it == NUM_ITERS - 1
            q3o, q3n = (q3a, q3b) if it % 2 == 0 else (q3b, q3a)
            # refresh intra-core group halos of the OLD q
            nc.sync.dma_start(q3o[21:126, 0:2, 0:NV], q3o[0:105, 14:16, 0:NV])
            nc.sync.dma_start(q3o[0:105, 16:18, 0:NV], q3o[21:126, 2:4, 0:NV])

            zs = z_banks()
            for (r0, r1, chunks) in HALVES:
                nr = r1 - r0
                for c in chunks:
                    sl = slice(c * CH, (c + 1) * CH)
                    nc.scalar.copy(zs[c][:], lg_t[:, sl])
                    nc.tensor.matmul(zs[c][:], mneg2_t[:],
                                     q3o[:, 2 + 2 * c:4 + 2 * c, 2:2 + W],
                                     start=False, stop=False,
                                     skip_group_check=True)
                for ki, (dy, dx) in enumerate(TAPS):
                    w4 = w_all[:].rearrange(
                        "p (q k t v) -> p q k t v", q=NQ, k=12,
                        v=SS)[:, :, ki]
                    for (qdy, qdx, wdy, wdx) in ((dy, dx, 0, 0),
                                                 (-dy, -dx, -dy, -dx)):
                        q_ap = q3o[:, 2 + qdy + r0:2 + qdy + r1,
                                   2 + qdx:2 + qdx + W].rearrange(
                            "p r (q x) -> p r q x", x=XW)
                        w_ap = w4[:, :, 2 + wdy + r0:2 + wdy + r1,
                                  2 + wdx:2 + wdx + XW].rearrange(
                            "p q r x -> p r q x")
                        t_t = t_pool.tile([P, nr * W], f16, tag=f"t{r0}",
                                          name="t_t")
                        t4 = t_t[:].rearrange("p (r q x) -> p r q x",
                                              q=NQ, x=XW)
                        nc.vector.tensor_mul(t4[:, :, :, :], q_ap, w_ap)
                        for c in chunks:
                            sl = slice((2 * c - r0) * W, (2 * c + 2 - r0) * W)
                            nc.tensor.matmul(
                                zs[c][:], mneg_t[:], t_t[:, sl],
                                start=False, stop=False,
                                skip_group_check=True)
                tail(zs, chunks, q3n, last=last)

        nc.sync.dma_start(qout_d.ap(), lg_t[:])

    _legalize_matmul_waits(nc, mybir)
    return nc


def _legalize_matmul_waits(nc, mybir, max_waits=2):
    """TRN2 ISA sync-wait structs hold few waits per instruction; codegen
    aborts on more. Move excess waits onto InstNoOps (1 wait each) inserted
    right before on the same engine."""
    cap = {}
    for f in nc.m.functions:
        for blk in f.blocks:
            insts = blk.instructions
            out = []
            changed = False
            for i in insts:
                si = getattr(i, "sync_info", None)
                eng = getattr(i, "engine", None)
                max_waits = cap.get(type(i).__name__, 1)
                if (si is not None and eng is not None
                        and len(si.on_wait) > max_waits):
                    waits = list(si.on_wait)
                    keep, move = [], []
                    for w in waits:
                        if "PE" in w.ant_name and len(keep) < max_waits:
                            keep.append(w)
                        else:
                            move.append(w)
                    while len(keep) < max_waits and move:
                        keep.append(move.pop())
                    nop_cap = cap.get("InstNoOp", 1)
                    while move:
                        grp, move = move[:nop_cap], move[nop_cap:]
                        nop = mybir.InstNoOp(
                            name=nc.get_next_instruction_name(),
                            engine=eng, ins=[], outs=[])
                        nop.sync_info = mybir.SyncInfo(on_wait=grp,
                                                       on_update=[])
                        out.append(nop)
                    i.sync_info = mybir.SyncInfo(
                        on_wait=keep, on_update=list(si.on_update))
                    changed = True
                out.append(i)
            if changed:
                blk.instructions = out


def _prep_shards(logits, img, compat):
    """Host-side shard prep -> list of 8 in_maps."""
    mneg = np.kron(np.eye(G), -compat.T.astype(np.float64)).astype(np.float16)
    mneg2 = ((1.0 + WC) * np.kron(np.eye(G), -compat.T.astype(np.float64))
             ).astype(np.float16)
    repl6 = np.zeros((44, 2 * P), np.float16)
    for g in range(G):
        repl6[g, g * C:(g + 1) * C] = 1.0          # rows 0-5 select block A
        repl6[G + g, P + g * C:P + (g + 1) * C] = 1.0  # rows 6-11, block B
    repl6[32:44] = repl6[0:12]                      # copy at base 32
    onesd = np.kron(np.eye(G), np.ones((C, 1))).astype(np.float16)
    bneg = np.kron(np.eye(G), -np.ones((1, C))).astype(np.float16)
    # rmask [(g,rgb,q), (g,q)]: sums rgb
    rmask = np.zeros((PRE_P, G * NQ), np.float16)
    for g in range(G):
        for rgb in range(3):
            for qt in range(NQ):
                rmask[(g * 3 + rgb) * NQ + qt, qt * G + g] = 1.0
    lns = np.tile(
        np.array([math.log(SW[2 + dy, 2 + dx]) for (dy, dx) in TAPS],
                 np.float32)[None, :], (44, 1))

    in_maps = []
    for core in range(8):
        b, j = divmod(core, 4)
        s = STARTS[j]
        lg = logits[b, :, s:s + 84, :].reshape(C, G, RG, W)
        lg = np.ascontiguousarray(
            lg.transpose(1, 0, 2, 3).reshape(P, F)).astype(np.float32)
        im = np.zeros((G, 3, NQ, IM_U, IM_V), np.float32)
        for g in range(G):
            rbase = s + g * RG - 4
            u0, u1 = max(0, -rbase), min(IM_U, H - rbase)
            for qt in range(NQ):
                cbase = qt * XW - 4
                v0, v1 = max(0, -cbase), min(IM_V, W - cbase)
                im[g, :, qt, u0:u1, v0:v1] = img[
                    b, :, rbase + u0:rbase + u1, cbase + v0:cbase + v1]
        im = im.reshape(PRE_P, IM_U * IM_V)
        in_maps.append({
            "lg": lg, "img": np.ascontiguousarray(im),
            "mneg": mneg, "mneg2": mneg2, "repl6": repl6, "onesd": onesd,
            "bneg": bneg, "rmask": rmask, "lns": lns,
        })
    return in_maps


def kernel(**inputs):
    logits = np.asarray(inputs["logits"], dtype=np.float32)
    img = np.asarray(inputs["img"], dtype=np.float32)
    compat = np.asarray(inputs["compat_mat"], dtype=np.float32)

    from concourse.bass_utils import run_bass_kernel_spmd

    if "nc" not in _BASS_CACHE:
        _BASS_CACHE["nc"] = _build_bass()
    nc = _BASS_CACHE["nc"]

    in_maps = _prep_shards(logits, img, compat)
    res = run_bass_kernel_spmd(nc, in_maps, core_ids=list(range(8)))
    _BASS_CACHE["last_result"] = res

    out = np.zeros((B, C, H, W), np.float32)
    for core in range(8):
        b, j = divmod(core, 4)
        s = STARTS[j]
        lo, hi = OWN[j]
        qc = res.results[core]["qout"].reshape(G, C, RG, W)
        qc = qc.transpose(1, 0, 2, 3).reshape(C, 84, W)
        out[b, :, s + lo:s + hi, :] = qc[:, lo:hi, :]
    return out
